# revision 28
# baseline (speedup 1.0000x reference)
"""Trainium2 Bass kernel for nn_Attentioncat (B=64, N=1024, NT=100, DIM=256,
KD=16, NH=8, D=64). Data-parallel over B across 8 NeuronCores (8 batches/core).

Math (per batch, derived from the reference):
  kv   = BN(x @ kv_w.T)            -> k [N,NH,KD], v [N,NH,D]
  q    = BN(text @ q_w.T) * KD^-.5    (host: tiny)
  attn = softmax_n(q.k + bias_table[idx])
  out  = BN(hswish([v | attn_feat | 0]) @ proj_w.T)

Device-side structure (transposed feature-major layout [f, n]). PE work is
minimized with fp8 DoubleRow matmuls (0.5 cyc/row) wherever precision allows;
the elementwise chain is spread over ACT / DVE / Pool:

  stage1: kv.T = W @ x.T via error-compensated fp8 hi/lo DoubleRow:
          8*W ~ wh+wl, x ~ xh+xl (each fp8e4); psum = wh.xh + wh.xl + wl.xh.
          k rows -> k_all bf16 (DVE, x1/8); v: v_sb = ps/8 + b [ACT],
          c3b = clip(v,-3,3)+3 [DVE 4x x2], u_v = c3b*v [TT: Pool/DVE]
  attn (7 tiles of 128 (h,t) rows):
      inject tiles: logits = DR(ident8,[bias_hi|bias_lo]) + qlhs.T@k_all;
          e = Exp(logits) w/ fused row-sum [ACT]
      m-route tiles: e0 = Exp(qk) [ACT]; e = e0*exp(bias) w/ row-sum [DVE]
      r = 1/s, rs = 32r [DVE]; a3 = 32*attn+96 [DVE 4x]
      u8 = (e.r)*a3 = 32*attn*(attn+3) -> fp8 [DVE fused stt / Pool TT]
      tile 6 (32 real rows + bias row at partition 32): bf16 [DVE]
  proj: ONE psum group per (pair,half): 5 bf16 matmuls ([v|tile6] @ wpv*2048)
        + 3 fp8 DoubleRow matmuls (u8 @ wpa8, scale 32*64); out = ACT copy
        with scale 1/2048. Proj bias enters via the constant-1.0 row planted
        once at partition 32 of the persistent tile-6 buffer.
"""

import os

import numpy as np
import ml_dtypes

import concourse.bacc as bacc
import concourse.bass as bass
import concourse.mybir as mybir
import concourse.tile as tile
from concourse.bass_utils import run_bass_kernel_spmd
from concourse.masks import make_identity

B, N, NT = 64, 1024, 100
DIM, KD, NH, D = 256, 16, 8, 64
DH = D * NH            # 512
NH_KD = KD * NH        # 128
H_KV = DH + NH_KD      # 640
EPS = 1e-5
NCORES = 8
BLOC = B // NCORES     # 8 batches per core

NT_PAD = 896           # 7 tiles of 128 rows for (h, t) pairs (800 real + pad)
N_ATILES = 7
N_FP8 = 6              # attn tiles 0..5 contract in fp8 DoubleRow
N_VTILES = DH // 128   # 4
ROW_ONE = 800          # (tile 6, partition 32): constant-1 row -> proj bias
U8S = 1.0              # u8 = u_a unscaled (small terms are noise-floor)
W8S = 64.0             # wpa8 = W8S * p_we.T / 6
OSC = U8S * W8S        # both proj groups accumulate at this scale
W1S = 8.0              # stage1 weights pre-scale before fp8 hi/lo split

M_ROUTE = (0,)         # attn tiles whose bias is applied as exp(bias) on DVE
POOL_U8 = (1, 2, 3)    # fp8 tiles whose u8 product runs on Pool (rest DVE)
POOL_UV = (0, 1)       # v-tiles whose u_v product runs on Pool (rest DVE)

f32 = mybir.dt.float32
bf16 = mybir.dt.bfloat16
f8e4 = mybir.dt.float8e4

AOP = mybir.AluOpType
DR = mybir.MatmulPerfMode.DoubleRow


def _fold_bn(w, g, b, m, v):
    s = (g / np.sqrt(v + EPS)).astype(np.float32)
    return (w * s[:, None]).astype(np.float32), (b - m * s).astype(np.float32)


def _build_program(loop_reps=1):
    """loop_reps>1 wraps the whole per-core body in a HW loop (timing only)."""
    nc = bacc.Bacc("TRN2", target_bir_lowering=False, debug=False)

    # DRAM tensors (per core). Weights replicated; x/out sharded over B.
    x8h_d = nc.dram_tensor("x8h", [BLOC, 2, 128, N], f8e4, kind="ExternalInput")
    x8l_d = nc.dram_tensor("x8l", [BLOC, 2, 128, N], f8e4, kind="ExternalInput")
    w8h_d = nc.dram_tensor("w8h", [128, 2, H_KV], f8e4, kind="ExternalInput")
    w8l_d = nc.dram_tensor("w8l", [128, 2, H_KV], f8e4, kind="ExternalInput")
    qlhs_d = nc.dram_tensor("qlhs", [128, NT_PAD], bf16, kind="ExternalInput")
    bgt8_d = nc.dram_tensor("bgt8", [128, N_ATILES, 2, N], f8e4, kind="ExternalInput")
    ebt_d = nc.dram_tensor("ebt", [128, max(len(M_ROUTE), 1), N], bf16,
                           kind="ExternalInput")
    wpv_d = nc.dram_tensor("wpv", [128, N_VTILES + 1, DIM], bf16, kind="ExternalInput")
    wpa8_d = nc.dram_tensor("wpa8", [128, N_FP8, DIM], f8e4, kind="ExternalInput")
    b1v_d = nc.dram_tensor("b1v", [128, N_VTILES], f32, kind="ExternalInput")
    out_d = nc.dram_tensor("out", [BLOC, N, DIM], f32, kind="ExternalOutput")

    with tile.TileContext(nc) as tc:
        with (
            tc.tile_pool(name="consts", bufs=1) as consts,
            tc.tile_pool(name="xtp", bufs=3) as xtp,
            tc.tile_pool(name="kallp", bufs=3) as kallp,
            tc.tile_pool(name="ep", bufs=3) as ep,
            tc.tile_pool(name="a3p", bufs=2) as a3p,
            tc.tile_pool(name="u8p", bufs=2) as u8p,
            tc.tile_pool(name="uvp", bufs=3) as uvp,
            tc.tile_pool(name="vtmp", bufs=6) as vtmp,
            tc.tile_pool(name="scol", bufs=10) as scol,
            tc.tile_pool(name="outp", bufs=2) as outp,
            tc.tile_pool(name="ps_sm", bufs=4, space="PSUM") as ps_sm,
            tc.tile_pool(name="ps_at", bufs=2, space="PSUM") as ps_at,
        ):
            # ---- constants ----
            ident = consts.tile([128, 128], f32, tag="ident")
            make_identity(nc, ident)
            # fp8 identity pair for the DoubleRow bias inject
            ident8_2 = consts.tile([128, 2, 128], f8e4, tag="ident8_2")
            nc.vector.tensor_copy(ident8_2[:, 0, :], ident)
            nc.vector.tensor_copy(ident8_2[:, 1, :], ident)

            w8h = consts.tile([128, 2, H_KV], f8e4, tag="w8h")
            w8l = consts.tile([128, 2, H_KV], f8e4, tag="w8l")
            nc.sync.dma_start(w8h, w8h_d.ap())
            nc.sync.dma_start(w8l, w8l_d.ap())
            b1v = consts.tile([128, N_VTILES], f32, tag="b1v")
            nc.sync.dma_start(b1v, b1v_d.ap())
            qlhs = consts.tile([128, NT_PAD], bf16, tag="qlhs")
            bgt8 = consts.tile([128, N_ATILES, 2, N], f8e4, tag="bgt8")
            ebt = consts.tile([128, max(len(M_ROUTE), 1), N], bf16, tag="ebt")
            wpv = consts.tile([128, N_VTILES + 1, DIM], bf16, tag="wpv")
            wpa8 = consts.tile([128, N_FP8, DIM], f8e4, tag="wpa8")

            # tile-6 u_a: persistent; partition 32 = 1.0 (proj bias row),
            # partitions 33.. = 0. Per-batch writes touch only rows 0..31.
            ua6 = consts.tile([128, N], bf16, tag="ua6")
            nc.vector.memset(ua6, 0.0)
            nc.vector.memset(ua6[32:33, :], 1.0)

            def emit_proj_pair(state, pair):
                b, u_v, u8, out_nat = state
                ps_o = ps_sm.tile([128, 512], f32, tag="ps")
                for half in range(2):
                    ntl = pair * 2 + half
                    nsl = slice(ntl * 128, (ntl + 1) * 128)
                    dsl = slice(half * DIM, (half + 1) * DIM)
                    for ft in range(N_VTILES + 1):
                        lhsT = u_v[:, ft, nsl] if ft < N_VTILES else ua6[:, nsl]
                        nc.tensor.matmul(
                            ps_o[:, dsl], lhsT=lhsT, rhs=wpv[:, ft, :],
                            start=(ft == 0), stop=False,
                        )
                    for j in range(N_FP8 // 2):
                        nc.tensor.matmul(
                            ps_o[:, dsl],
                            lhsT=u8[:, 2 * j : 2 * j + 2, nsl],
                            rhs=wpa8[:, 2 * j : 2 * j + 2, :],
                            start=False, stop=(j == N_FP8 // 2 - 1),
                            perf_mode=DR,
                        )
                osl = out_nat[:, pair * 2 : pair * 2 + 2, :]
                nc.scalar.activation(
                    osl, ps_o, mybir.ActivationFunctionType.Copy,
                    scale=1.0 / OSC,
                )
                if pair == 3:
                    nc.sync.dma_start(
                        out_d.ap()[b].rearrange("(t p) d -> p t d", p=128),
                        out_nat,
                    )

            prev = None
            import contextlib
            loop_cm = (
                tc.For_i(
                    0, loop_reps, 1,
                    hint_engines=(
                        mybir.EngineType.PE,
                        mybir.EngineType.DVE,
                        mybir.EngineType.Activation,
                        mybir.EngineType.Pool,
                    ),
                )
                if loop_reps > 1
                else contextlib.nullcontext()
            )
            with loop_cm:
              xts = {}

              def load_xt(bb):
                  th = xtp.tile([128, 2, N], f8e4, tag="xh", name=f"x8h_{bb}")
                  tl = xtp.tile([128, 2, N], f8e4, tag="xl", name=f"x8l_{bb}")
                  nc.sync.dma_start(th, x8h_d.ap()[bb].rearrange("t p n -> p t n"))
                  nc.sync.dma_start(tl, x8l_d.ap()[bb].rearrange("t p n -> p t n"))
                  xts[bb] = (th, tl)

              cur_s1 = None
              for b in range(BLOC):
                  # ---- prefetch next batch's x; batch 0 loads its own ----
                  if b == 0:
                      load_xt(0)
                  if b + 1 < BLOC:
                      load_xt(b + 1)
                  xh, xl = xts.pop(b)
                  if cur_s1 is None:
                      k0 = kallp.tile([128, N], bf16, tag="k_all", name="k_all0")
                      uv0 = uvp.tile([128, N_VTILES, N], bf16, tag="u_v",
                                     name="u_v0")
                      cur_s1 = (k0, uv0)
                  if b == 0:
                      nc.sync.dma_start(qlhs, qlhs_d.ap())
                      nc.gpsimd.dma_start(bgt8, bgt8_d.ap())
                      nc.gpsimd.dma_start(ebt, ebt_d.ap())
                  elif b == 1:
                      nc.gpsimd.dma_start(wpv, wpv_d.ap())
                      nc.gpsimd.dma_start(wpa8, wpa8_d.ap())

                  # ---- stage1 chunk emitters (kv.T = W @ x.T, fp8 hi/lo
                  # DoubleRow). Chunk 0 (the k rows) for batch b ran during
                  # iter b-1; the rest are interleaved into this iter's attn
                  # phase to keep every engine streaming.
                  def emit_s1_chunk(xh, xl, k_all, u_v, mt, nch):
                      msl = slice(mt * 128, (mt + 1) * 128)
                      ps_kv = ps_sm.tile([128, 512], f32, tag="ps")
                      nsl = slice(nch * 512, (nch + 1) * 512)
                      for lhsT, rhs, st, sp in (
                          (w8h[:, :, msl], xh[:, :, nsl], True, False),
                          (w8h[:, :, msl], xl[:, :, nsl], False, False),
                          (w8l[:, :, msl], xh[:, :, nsl], False, True),
                      ):
                          nc.tensor.matmul(
                              ps_kv, lhsT=lhsT, rhs=rhs,
                              start=st, stop=sp, perf_mode=DR,
                          )
                      if mt == 0:
                          nc.vector.tensor_scalar(
                              k_all[:, nsl], ps_kv, 1.0 / W1S, None,
                              op0=AOP.mult,
                          )
                      else:
                          vt = mt - 1
                          v_sb = vtmp.tile([128, 512], bf16, tag="v_sb")
                          nc.scalar.activation(
                              v_sb, ps_kv,
                              mybir.ActivationFunctionType.Identity,
                              bias=b1v[:, vt : vt + 1], scale=1.0 / W1S,
                          )
                          c3b = vtmp.tile([128, 512], bf16, tag="c3b")
                          # c3b = clip(v,-3,3)+3 = clip(v+3,0,6)
                          nc.vector.tensor_scalar(
                              c3b, v_sb, -3.0, 3.0,
                              op0=AOP.max, op1=AOP.min,
                          )
                          nc.vector.tensor_scalar(
                              c3b, c3b, 3.0, None, op0=AOP.add,
                          )
                          # u_v = c3b * v = 6*hswish(v)
                          eng = nc.gpsimd if vt in POOL_UV else nc.vector
                          eng.tensor_tensor(
                              u_v[:, vt, nsl], c3b, v_sb, op=AOP.mult,
                          )

                  k_all, u_v = cur_s1
                  for mt in range(2):
                      for nch in range(2):
                          emit_s1_chunk(xh, xl, k_all, u_v, mt, nch)

                  # stage1 tail chunks (this batch) + next batch's k rows are
                  # spread across the attn phase below.
                  s1_tail = [(xh, xl, k_all, u_v, mt, nch)
                             for mt in range(2, 5) for nch in range(2)]
                  if b + 1 < BLOC:
                      nk = kallp.tile([128, N], bf16, tag="k_all",
                                      name=f"k_all{b + 1}")
                      nuv = uvp.tile([128, N_VTILES, N], bf16, tag="u_v",
                                     name=f"u_v{b + 1}")
                      nxt_s1 = (nk, nuv)
                      nxh, nxl = xts[b + 1]
                  else:
                      nxt_s1 = None
                  # slots: after attn tiles 1..6 -> one stage1 chunk each,
                  # tail first, then next batch's two k chunks at the end
                  s1_slots = {}
                  pending = list(s1_tail)
                  if nxt_s1 is not None:
                      pending += [(nxh, nxl, nxt_s1[0], nxt_s1[1], 0, nch)
                                  for nch in range(2)]
                  for i, item in enumerate(pending):
                      s1_slots.setdefault(min(1 + i // 2, 6), []).append(item)

                  # ---- attention, with the previous batch's proj pairs and
                  # stage1 chunks interleaved to fill PE bubbles ----
                  if prev is not None:
                      out_nat = outp.tile([128, 8, DIM], f32, tag="out_nat")
                      pstate = (*prev, out_nat)
                  proj_after = {0: 0, 1: 1, 2: 2, 3: 3}
                  u8 = u8p.tile([128, N_FP8, N], f8e4, tag="u8")
                  for at in range(N_ATILES):
                      if prev is not None and at in proj_after:
                          emit_proj_pair(pstate, proj_after[at])
                      for item in s1_slots.get(at, ()):
                          emit_s1_chunk(*item)
                      ps_l = ps_at.tile([128, N], f32, tag="ps_l")
                      for nch in range(2):
                          nsl = slice(nch * 512, (nch + 1) * 512)
                          if at in M_ROUTE:
                              nc.tensor.matmul(
                                  ps_l[:, nsl],
                                  lhsT=qlhs[:, at * 128 : (at + 1) * 128],
                                  rhs=k_all[:, nsl],
                                  start=True, stop=True,
                              )
                          else:
                              nc.tensor.matmul(
                                  ps_l[:, nsl], lhsT=ident8_2,
                                  rhs=bgt8[:, at, :, nsl],
                                  start=True, stop=False, perf_mode=DR,
                              )
                              nc.tensor.matmul(
                                  ps_l[:, nsl],
                                  lhsT=qlhs[:, at * 128 : (at + 1) * 128],
                                  rhs=k_all[:, nsl],
                                  start=False, stop=True,
                              )
                      s_c = scol.tile([128, 1], f32, tag="s_c")
                      e = ep.tile([128, N], bf16, tag="e")
                      if at in M_ROUTE:
                          e0 = ep.tile([128, N], bf16, tag="e0")
                          nc.scalar.activation(
                              e0, ps_l, mybir.ActivationFunctionType.Exp,
                          )
                          # e = e0 * exp(bias); fused row-sum
                          nc.vector.scalar_tensor_tensor(
                              e, e0, 1.0, ebt[:, M_ROUTE.index(at), :],
                              op0=AOP.mult, op1=AOP.mult, accum_out=s_c,
                          )
                      else:
                          nc.scalar.activation(
                              e, ps_l, mybir.ActivationFunctionType.Exp,
                              accum_out=s_c,
                          )
                      r_c = scol.tile([128, 1], f32, tag="r_c")
                      nc.vector.reciprocal(r_c, s_c)
                      if at < N_FP8:
                          # attn3 = attn + 3  (4x)
                          a3 = a3p.tile([128, N], bf16, tag="a3")
                          nc.vector.tensor_scalar(
                              a3, e, r_c, 3.0, op0=AOP.mult, op1=AOP.add,
                          )
                          if at in POOL_U8:
                              at_t = a3p.tile([128, N], bf16, tag="at_t")
                              nc.vector.tensor_scalar(
                                  at_t, e, r_c, None, op0=AOP.mult,
                              )
                              # u8 = attn * attn3f  [Pool TT, fp8 out]
                              nc.gpsimd.tensor_tensor(
                                  u8[:, at, :], at_t, a3, op=AOP.mult,
                              )
                          else:
                              # u8 = (e*r)*attn3f  [DVE fused stt, fp8 out]
                              nc.vector.scalar_tensor_tensor(
                                  u8[:, at, :], e, r_c, a3,
                                  op0=AOP.mult, op1=AOP.mult,
                              )
                      else:
                          # tile 6: rows 0..31 are feature rows 768..799;
                          # partition 32 is the preset proj-bias row.
                          at_t = a3p.tile([128, N], bf16, tag="at_t")
                          a3 = a3p.tile([128, N], bf16, tag="a3")
                          nc.vector.tensor_scalar(
                              at_t[0:32, :], e[0:32, :], r_c[0:32, :], None,
                              op0=AOP.mult,
                          )
                          nc.vector.tensor_scalar(
                              a3[0:32, :], e[0:32, :], r_c[0:32, :], 3.0,
                              op0=AOP.mult, op1=AOP.add,
                          )
                          nc.vector.tensor_tensor(
                              ua6[0:32, :], at_t[0:32, :], a3[0:32, :],
                              op=AOP.mult,
                          )

                  prev = (b, u_v, u8)
                  cur_s1 = nxt_s1

              out_nat = outp.tile([128, 8, DIM], f32, tag="out_nat")
              pstate = (*prev, out_nat)
              for pair in range(4):
                  emit_proj_pair(pstate, pair)

    nc.compile()
    return nc


_PROGRAM_CACHE = {}


def _get_program():
    if "nc" not in _PROGRAM_CACHE:
        _PROGRAM_CACHE["nc"] = _build_program()
    return _PROGRAM_CACHE["nc"]


def _f8(x):
    return np.asarray(x, dtype=ml_dtypes.float8_e4m3)


def _prepare_host_inputs(x, text, kv_w, kv_g, kv_b, kv_m, kv_v,
                         q_w, q_g, q_b, q_m, q_v,
                         proj_w, proj_g, proj_b, proj_m, proj_v,
                         biases, H, W):
    H, W = int(H), int(W)
    scale = KD ** -0.5

    kv_we, kv_be = _fold_bn(np.asarray(kv_w), np.asarray(kv_g), np.asarray(kv_b),
                            np.asarray(kv_m), np.asarray(kv_v))
    q_we, q_be = _fold_bn(np.asarray(q_w), np.asarray(q_g), np.asarray(q_b),
                          np.asarray(q_m), np.asarray(q_v))
    p_we, p_be = _fold_bn(np.asarray(proj_w), np.asarray(proj_g), np.asarray(proj_b),
                          np.asarray(proj_m), np.asarray(proj_v))

    # kv feature permutation: k rows first (h-major kd), then v rows (h-major d)
    k_src = np.array([h * (KD + D) + j for h in range(NH) for j in range(KD)])
    v_src = np.array([h * (KD + D) + KD + d for h in range(NH) for d in range(D)])
    perm = np.concatenate([k_src, v_src])
    w1 = kv_we[perm] * W1S                # [640, 256], pre-scaled
    b1 = kv_be[perm]                      # [640]
    w1t = np.ascontiguousarray(
        w1.T.reshape(2, 128, H_KV).transpose(1, 0, 2)
    ).astype(np.float32)                  # [128, 2, 640]
    w8h_host = _f8(w1t)
    w8l_host = _f8(w1t - w8h_host.astype(np.float32))
    b1v_host = np.ascontiguousarray(
        b1[NH_KD:].reshape(N_VTILES, 128).T
    ).astype(np.float32)                  # [128, 4]

    # q on host (tiny), scaled
    q = (np.asarray(text, np.float32) @ q_we.T + q_be).reshape(NT, NH, KD)
    q = (q * scale).astype(np.float32)

    # block-diagonal lhsT for the attn matmul: [128 (h,kd), 896 (h,t)]
    qlhs_host = np.zeros((128, NT_PAD), np.float32)
    rows = np.arange(NH * NT)
    hh, tt = rows // NT, rows % NT
    for kd in range(KD):
        qlhs_host[hh * KD + kd, rows] = q[tt, hh, kd]
    qlhs_host = qlhs_host.astype(ml_dtypes.bfloat16)

    # bias table[(h,t), n] = biases[h, idx[t, n]] + q~[t,h] . b1_k[h]
    t_i = np.arange(NT)
    n_i = np.arange(N)
    p1x, p1y = t_i // 100, t_i % 100
    p2x, p2y = n_i // W, n_i % W
    idx = (np.abs(p1x[:, None] - p2x[None, :]) * 100
           + np.abs(p1y[:, None] - p2y[None, :]))        # [100, N]
    bias_g = np.asarray(biases, np.float32)[:, idx]       # [NH, 100, N]
    b1k = b1[:NH_KD].reshape(NH, KD)                      # [8, 16]
    cq = np.einsum("thk,hk->ht", q, b1k)                  # [8, 100]
    bg2_full = np.zeros((NT_PAD, N), np.float32)
    bg2_full[: NH * NT] = (bias_g + cq[:, :, None]).reshape(NH * NT, N)
    bg2_t = np.ascontiguousarray(
        bg2_full.reshape(N_ATILES, 128, N).transpose(1, 0, 2)
    )                                                     # [128, 7, N]
    # hi/lo fp8 split for the DoubleRow inject; exp() table for m-route tiles
    bgh = _f8(bg2_t)
    bgl = _f8(bg2_t - bgh.astype(np.float32))
    bgt8_host = np.ascontiguousarray(
        np.stack([bgh, bgl], axis=2)
    )                                                     # [128, 7, 2, N] fp8
    if M_ROUTE:
        ebt_host = np.ascontiguousarray(
            np.exp(bg2_t[:, list(M_ROUTE), :])
        ).astype(ml_dtypes.bfloat16)
    else:
        ebt_host = np.zeros((128, 1, N), ml_dtypes.bfloat16)

    # proj weights.
    # bf16 group (x OSC): v rows (u_v = 6*hswish(v), /6) then attn tile 6
    # (feature rows 768..799 at partitions 0..31, proj bias at partition 32).
    wpv_host = np.zeros((128, N_VTILES + 1, DIM), np.float32)
    wpv_host[:, :N_VTILES, :] = (
        p_we[:, :DH].T.reshape(N_VTILES, 128, DIM).transpose(1, 0, 2)
        * (OSC / 6.0)
    )
    wpv_host[0:32, N_VTILES, :] = p_we[:, DH + 768 : DH + 800].T * (OSC / 6.0)
    wpv_host[32, N_VTILES, :] = p_be * OSC
    wpv_host = wpv_host.astype(ml_dtypes.bfloat16)

    # fp8 group: attn feature rows 0..767; u8 = U8S*u_a, wpa8 = W8S*w/6
    wpa8_host = _f8(
        np.ascontiguousarray(
            (p_we[:, DH : DH + 768].T * (W8S / 6.0))
            .reshape(N_FP8, 128, DIM).transpose(1, 0, 2)
        )
    )                                                     # [128, 6, 256]

    return {
        "w8h": w8h_host,
        "w8l": w8l_host,
        "qlhs": qlhs_host,
        "bgt8": bgt8_host,
        "ebt": ebt_host,
        "wpv": wpv_host,
        "wpa8": wpa8_host,
        "b1v": b1v_host,
    }


def kernel(**inputs):
    x = np.asarray(inputs["x"], np.float32)
    consts = _prepare_host_inputs(**inputs)
    # x.T per batch, f-major tiles, fp8 hi/lo: [B, 2, 128, N]
    xt_all = np.ascontiguousarray(x.transpose(0, 2, 1).reshape(B, 2, 128, N))
    x8h_all = _f8(xt_all)
    x8l_all = _f8(xt_all - x8h_all.astype(np.float32))

    nc = _get_program()
    in_maps = []
    for c in range(NCORES):
        m = dict(consts)
        m["x8h"] = np.ascontiguousarray(x8h_all[c * BLOC : (c + 1) * BLOC])
        m["x8l"] = np.ascontiguousarray(x8l_all[c * BLOC : (c + 1) * BLOC])
        in_maps.append(m)

    trace = bool(int(os.environ.get("KERNEL_TRACE", "0")))
    res = None
    last_err = None
    for _attempt in range(3):
        try:
            res = run_bass_kernel_spmd(
                nc, in_maps, core_ids=list(range(NCORES)), trace=trace
            )
            break
        except Exception as e:  # transient NRT device wedge: retry
            last_err = e
            trace = False  # trace path may be unavailable (no ntff hook)
    if res is None:
        raise last_err
    if trace and res.exec_time_ns is not None:
        print(f"HW exec time: {res.exec_time_ns} ns")
        if res.instructions_and_trace is not None:
            print(f"trace: {res.instructions_and_trace[1]}")
    out = np.concatenate([r["out"] for r in res.results], axis=0)
    return out


# revision 29
# speedup vs baseline: 1.1373x; 1.1373x over previous
"""Trainium2 Bass kernel for nn_Attentioncat (B=64, N=1024, NT=100, DIM=256,
KD=16, NH=8, D=64). Data-parallel over B across 8 NeuronCores (8 batches/core).

Math (per batch, derived from the reference):
  kv   = BN(x @ kv_w.T)            -> k [N,NH,KD], v [N,NH,D]
  q    = BN(text @ q_w.T) * KD^-.5    (host: tiny)
  attn = softmax_n(q.k + bias_table[idx])
  out  = BN(hswish([v | attn_feat | 0]) @ proj_w.T)

Device-side structure (transposed feature-major layout [f, n]). PE work is
minimized with fp8 DoubleRow matmuls (0.5 cyc/row) wherever precision allows;
the elementwise chain is spread over ACT / DVE / Pool:

  stage1: kv.T = W @ x.T via error-compensated fp8 hi/lo DoubleRow:
          8*W ~ wh+wl, x ~ xh+xl (each fp8e4); psum = wh.xh + wh.xl + wl.xh.
          k rows -> k_all bf16 (DVE, x1/8); v: v_sb = ps/8 + b [ACT],
          c3b = clip(v,-3,3)+3 [DVE 4x x2], u_v = c3b*v [TT: Pool/DVE]
  attn (7 tiles of 128 (h,t) rows):
      inject tiles: logits = DR(ident8,[bias_hi|bias_lo]) + qlhs.T@k_all;
          e = Exp(logits) w/ fused row-sum [ACT]
      m-route tiles: e0 = Exp(qk) [ACT]; e = e0*exp(bias) w/ row-sum [DVE]
      r = 1/s, rs = 32r [DVE]; a3 = 32*attn+96 [DVE 4x]
      u8 = (e.r)*a3 = 32*attn*(attn+3) -> fp8 [DVE fused stt / Pool TT]
      tile 6 (32 real rows + bias row at partition 32): bf16 [DVE]
  proj: ONE psum group per (pair,half): 5 bf16 matmuls ([v|tile6] @ wpv*2048)
        + 3 fp8 DoubleRow matmuls (u8 @ wpa8, scale 32*64); out = ACT copy
        with scale 1/2048. Proj bias enters via the constant-1.0 row planted
        once at partition 32 of the persistent tile-6 buffer.
"""

import os

import numpy as np
import ml_dtypes

import concourse.bacc as bacc
import concourse.bass as bass
import concourse.mybir as mybir
import concourse.tile as tile
from concourse.bass_utils import run_bass_kernel_spmd
from concourse.masks import make_identity

B, N, NT = 64, 1024, 100
DIM, KD, NH, D = 256, 16, 8, 64
DH = D * NH            # 512
NH_KD = KD * NH        # 128
H_KV = DH + NH_KD      # 640
EPS = 1e-5
NCORES = 8
BLOC = B // NCORES     # 8 batches per core

NT_PAD = 896           # 7 tiles of 128 rows for (h, t) pairs (800 real + pad)
N_ATILES = 7
N_FP8 = 6              # attn tiles 0..5 contract in fp8 DoubleRow
N_VTILES = DH // 128   # 4
ROW_ONE = 800          # (tile 6, partition 32): constant-1 row -> proj bias
U8S = 1.0              # u8 = u_a unscaled (small terms are noise-floor)
W8S = 64.0             # wpa8 = W8S * p_we.T / 6
OSC = U8S * W8S        # both proj groups accumulate at this scale
W1S = 8.0              # stage1 weights pre-scale before fp8 hi/lo split

M_ROUTE = (0,)         # attn tiles whose bias is applied as exp(bias) on DVE
POOL_U8 = ()           # fp8 tiles whose u8 product runs on Pool (rest DVE)
POOL_UV = ()           # v-tiles whose u_v product runs on Pool (rest DVE)

f32 = mybir.dt.float32
bf16 = mybir.dt.bfloat16
f8e4 = mybir.dt.float8e4

AOP = mybir.AluOpType
DR = mybir.MatmulPerfMode.DoubleRow


def _fold_bn(w, g, b, m, v):
    s = (g / np.sqrt(v + EPS)).astype(np.float32)
    return (w * s[:, None]).astype(np.float32), (b - m * s).astype(np.float32)


def _build_program(loop_reps=1):
    """loop_reps>1 wraps the whole per-core body in a HW loop (timing only)."""
    nc = bacc.Bacc("TRN2", target_bir_lowering=False, debug=False)

    # DRAM tensors (per core). Weights replicated; x/out sharded over B.
    x8h_d = nc.dram_tensor("x8h", [BLOC, 2, 128, N], f8e4, kind="ExternalInput")
    x8l_d = nc.dram_tensor("x8l", [BLOC, 2, 128, N], f8e4, kind="ExternalInput")
    w8h_d = nc.dram_tensor("w8h", [128, 2, H_KV], f8e4, kind="ExternalInput")
    w8l_d = nc.dram_tensor("w8l", [128, 2, H_KV], f8e4, kind="ExternalInput")
    qlhs_d = nc.dram_tensor("qlhs", [128, NT_PAD], bf16, kind="ExternalInput")
    bgt8_d = nc.dram_tensor("bgt8", [128, N_ATILES, 2, N], f8e4, kind="ExternalInput")
    ebt_d = nc.dram_tensor("ebt", [128, max(len(M_ROUTE), 1), N], bf16,
                           kind="ExternalInput")
    wpv_d = nc.dram_tensor("wpv", [128, N_VTILES + 1, DIM], bf16, kind="ExternalInput")
    wpa8_d = nc.dram_tensor("wpa8", [128, N_FP8, DIM], f8e4, kind="ExternalInput")
    b1v_d = nc.dram_tensor("b1v", [128, N_VTILES], f32, kind="ExternalInput")
    out_d = nc.dram_tensor("out", [BLOC, N, DIM], f32, kind="ExternalOutput")

    with tile.TileContext(nc) as tc:
        with (
            tc.tile_pool(name="consts", bufs=1) as consts,
            tc.tile_pool(name="xtp", bufs=3) as xtp,
            tc.tile_pool(name="kallp", bufs=3) as kallp,
            tc.tile_pool(name="ep", bufs=3) as ep,
            tc.tile_pool(name="a3p", bufs=2) as a3p,
            tc.tile_pool(name="u8p", bufs=2) as u8p,
            tc.tile_pool(name="uvp", bufs=3) as uvp,
            tc.tile_pool(name="vtmp", bufs=6) as vtmp,
            tc.tile_pool(name="scol", bufs=10) as scol,
            tc.tile_pool(name="outp", bufs=2) as outp,
            tc.tile_pool(name="ps_sm", bufs=4, space="PSUM") as ps_sm,
            tc.tile_pool(name="ps_at", bufs=2, space="PSUM") as ps_at,
        ):
            # ---- constants ----
            ident = consts.tile([128, 128], f32, tag="ident")
            make_identity(nc, ident)
            # fp8 identity pair for the DoubleRow bias inject
            ident8_2 = consts.tile([128, 2, 128], f8e4, tag="ident8_2")
            nc.vector.tensor_copy(ident8_2[:, 0, :], ident)
            nc.vector.tensor_copy(ident8_2[:, 1, :], ident)

            w8h = consts.tile([128, 2, H_KV], f8e4, tag="w8h")
            w8l = consts.tile([128, 2, H_KV], f8e4, tag="w8l")
            nc.sync.dma_start(w8h, w8h_d.ap())
            nc.sync.dma_start(w8l, w8l_d.ap())
            b1v = consts.tile([128, N_VTILES], f32, tag="b1v")
            nc.sync.dma_start(b1v, b1v_d.ap())
            qlhs = consts.tile([128, NT_PAD], bf16, tag="qlhs")
            bgt8 = consts.tile([128, N_ATILES, 2, N], f8e4, tag="bgt8")
            ebt = consts.tile([128, max(len(M_ROUTE), 1), N], bf16, tag="ebt")
            wpv = consts.tile([128, N_VTILES + 1, DIM], bf16, tag="wpv")
            wpa8 = consts.tile([128, N_FP8, DIM], f8e4, tag="wpa8")

            # tile-6 u_a: persistent; partition 32 = 1.0 (proj bias row),
            # partitions 33.. = 0. Per-batch writes touch only rows 0..31.
            ua6 = consts.tile([128, N], bf16, tag="ua6")
            nc.vector.memset(ua6, 0.0)
            nc.vector.memset(ua6[32:33, :], 1.0)

            def emit_proj_pair(state, pair):
                b, u_v, u8, out_nat = state
                ps_o = ps_sm.tile([128, 512], f32, tag="ps")
                for half in range(2):
                    ntl = pair * 2 + half
                    nsl = slice(ntl * 128, (ntl + 1) * 128)
                    dsl = slice(half * DIM, (half + 1) * DIM)
                    for ft in range(N_VTILES + 1):
                        lhsT = u_v[:, ft, nsl] if ft < N_VTILES else ua6[:, nsl]
                        nc.tensor.matmul(
                            ps_o[:, dsl], lhsT=lhsT, rhs=wpv[:, ft, :],
                            start=(ft == 0), stop=False,
                        )
                    for j in range(N_FP8 // 2):
                        nc.tensor.matmul(
                            ps_o[:, dsl],
                            lhsT=u8[:, 2 * j : 2 * j + 2, nsl],
                            rhs=wpa8[:, 2 * j : 2 * j + 2, :],
                            start=False, stop=(j == N_FP8 // 2 - 1),
                            perf_mode=DR,
                        )
                osl = out_nat[:, pair * 2 : pair * 2 + 2, :]
                nc.scalar.activation(
                    osl, ps_o, mybir.ActivationFunctionType.Copy,
                    scale=1.0 / OSC,
                )
                if pair == 3:
                    nc.sync.dma_start(
                        out_d.ap()[b].rearrange("(t p) d -> p t d", p=128),
                        out_nat,
                    )

            prev = None
            import contextlib
            loop_cm = (
                tc.For_i(
                    0, loop_reps, 1,
                    hint_engines=(
                        mybir.EngineType.PE,
                        mybir.EngineType.DVE,
                        mybir.EngineType.Activation,
                        mybir.EngineType.Pool,
                    ),
                )
                if loop_reps > 1
                else contextlib.nullcontext()
            )
            with loop_cm:
              xts = {}

              def load_xt(bb):
                  th = xtp.tile([128, 2, N], f8e4, tag="xh", name=f"x8h_{bb}")
                  tl = xtp.tile([128, 2, N], f8e4, tag="xl", name=f"x8l_{bb}")
                  nc.sync.dma_start(th, x8h_d.ap()[bb].rearrange("t p n -> p t n"))
                  nc.sync.dma_start(tl, x8l_d.ap()[bb].rearrange("t p n -> p t n"))
                  xts[bb] = (th, tl)

              cur_s1 = None
              for b in range(BLOC):
                  # ---- prefetch next batch's x; batch 0 loads its own ----
                  if b == 0:
                      load_xt(0)
                  if b + 1 < BLOC:
                      load_xt(b + 1)
                  xh, xl = xts.pop(b)
                  if cur_s1 is None:
                      k0 = kallp.tile([128, N], bf16, tag="k_all", name="k_all0")
                      uv0 = uvp.tile([128, N_VTILES, N], bf16, tag="u_v",
                                     name="u_v0")
                      cur_s1 = (k0, uv0)
                  if b == 0:
                      nc.sync.dma_start(qlhs, qlhs_d.ap())
                      nc.gpsimd.dma_start(bgt8, bgt8_d.ap())
                      nc.gpsimd.dma_start(ebt, ebt_d.ap())
                  elif b == 1:
                      nc.gpsimd.dma_start(wpv, wpv_d.ap())
                      nc.gpsimd.dma_start(wpa8, wpa8_d.ap())

                  # ---- stage1 chunk emitters (kv.T = W @ x.T, fp8 hi/lo
                  # DoubleRow). Chunk 0 (the k rows) for batch b ran during
                  # iter b-1; the rest are interleaved into this iter's attn
                  # phase to keep every engine streaming.
                  def emit_s1_chunk(xh, xl, k_all, u_v, mt, nch):
                      msl = slice(mt * 128, (mt + 1) * 128)
                      ps_kv = ps_sm.tile([128, 512], f32, tag="ps")
                      nsl = slice(nch * 512, (nch + 1) * 512)
                      for lhsT, rhs, st, sp in (
                          (w8h[:, :, msl], xh[:, :, nsl], True, False),
                          (w8h[:, :, msl], xl[:, :, nsl], False, False),
                          (w8l[:, :, msl], xh[:, :, nsl], False, True),
                      ):
                          nc.tensor.matmul(
                              ps_kv, lhsT=lhsT, rhs=rhs,
                              start=st, stop=sp, perf_mode=DR,
                          )
                      if mt == 0:
                          nc.vector.tensor_scalar(
                              k_all[:, nsl], ps_kv, 1.0 / W1S, None,
                              op0=AOP.mult,
                          )
                      else:
                          vt = mt - 1
                          v_sb = vtmp.tile([128, 512], bf16, tag="v_sb")
                          nc.scalar.activation(
                              v_sb, ps_kv,
                              mybir.ActivationFunctionType.Identity,
                              bias=b1v[:, vt : vt + 1], scale=1.0 / W1S,
                          )
                          c3b = vtmp.tile([128, 512], bf16, tag="c3b")
                          # c3b = clip(v,-3,3)+3 = clip(v+3,0,6)
                          nc.vector.tensor_scalar(
                              c3b, v_sb, -3.0, 3.0,
                              op0=AOP.max, op1=AOP.min,
                          )
                          nc.vector.tensor_scalar(
                              c3b, c3b, 3.0, None, op0=AOP.add,
                          )
                          # u_v = c3b * v = 6*hswish(v)
                          eng = nc.gpsimd if vt in POOL_UV else nc.vector
                          eng.tensor_tensor(
                              u_v[:, vt, nsl], c3b, v_sb, op=AOP.mult,
                          )

                  k_all, u_v = cur_s1
                  for mt in range(2):
                      for nch in range(2):
                          emit_s1_chunk(xh, xl, k_all, u_v, mt, nch)

                  # stage1 tail chunks (this batch) + next batch's k rows are
                  # spread across the attn phase below.
                  s1_tail = [(xh, xl, k_all, u_v, mt, nch)
                             for mt in range(2, 5) for nch in range(2)]
                  if b + 1 < BLOC:
                      nk = kallp.tile([128, N], bf16, tag="k_all",
                                      name=f"k_all{b + 1}")
                      nuv = uvp.tile([128, N_VTILES, N], bf16, tag="u_v",
                                     name=f"u_v{b + 1}")
                      nxt_s1 = (nk, nuv)
                      nxh, nxl = xts[b + 1]
                  else:
                      nxt_s1 = None
                  # slots: after attn tiles 1..6 -> one stage1 chunk each,
                  # tail first, then next batch's two k chunks at the end
                  s1_slots = {}
                  pending = list(s1_tail)
                  if nxt_s1 is not None:
                      pending += [(nxh, nxl, nxt_s1[0], nxt_s1[1], 0, nch)
                                  for nch in range(2)]
                  for i, item in enumerate(pending):
                      s1_slots.setdefault(min(1 + i // 2, 6), []).append(item)

                  # ---- attention, with the previous batch's proj pairs and
                  # stage1 chunks interleaved to fill PE bubbles ----
                  if prev is not None:
                      out_nat = outp.tile([128, 8, DIM], f32, tag="out_nat")
                      pstate = (*prev, out_nat)
                  proj_after = {0: 0, 1: 1, 2: 2, 3: 3}
                  u8 = u8p.tile([128, N_FP8, N], f8e4, tag="u8")
                  for at in range(N_ATILES):
                      if prev is not None and at in proj_after:
                          emit_proj_pair(pstate, proj_after[at])
                      for item in s1_slots.get(at, ()):
                          emit_s1_chunk(*item)
                      ps_l = ps_at.tile([128, N], f32, tag="ps_l")
                      for nch in range(2):
                          nsl = slice(nch * 512, (nch + 1) * 512)
                          if at in M_ROUTE:
                              nc.tensor.matmul(
                                  ps_l[:, nsl],
                                  lhsT=qlhs[:, at * 128 : (at + 1) * 128],
                                  rhs=k_all[:, nsl],
                                  start=True, stop=True,
                              )
                          else:
                              nc.tensor.matmul(
                                  ps_l[:, nsl], lhsT=ident8_2,
                                  rhs=bgt8[:, at, :, nsl],
                                  start=True, stop=False, perf_mode=DR,
                              )
                              nc.tensor.matmul(
                                  ps_l[:, nsl],
                                  lhsT=qlhs[:, at * 128 : (at + 1) * 128],
                                  rhs=k_all[:, nsl],
                                  start=False, stop=True,
                              )
                      s_c = scol.tile([128, 1], f32, tag="s_c")
                      e = ep.tile([128, N], bf16, tag="e")
                      if at in M_ROUTE:
                          e0 = ep.tile([128, N], bf16, tag="e0")
                          nc.scalar.activation(
                              e0, ps_l, mybir.ActivationFunctionType.Exp,
                          )
                          # e = e0 * exp(bias); fused row-sum
                          nc.vector.scalar_tensor_tensor(
                              e, e0, 1.0, ebt[:, M_ROUTE.index(at), :],
                              op0=AOP.mult, op1=AOP.mult, accum_out=s_c,
                          )
                      else:
                          nc.scalar.activation(
                              e, ps_l, mybir.ActivationFunctionType.Exp,
                              accum_out=s_c,
                          )
                      r_c = scol.tile([128, 1], f32, tag="r_c")
                      nc.vector.reciprocal(r_c, s_c)
                      if at < N_FP8:
                          # attn3 = attn + 3  (4x)
                          a3 = a3p.tile([128, N], bf16, tag="a3")
                          nc.vector.tensor_scalar(
                              a3, e, r_c, 3.0, op0=AOP.mult, op1=AOP.add,
                          )
                          if at in POOL_U8:
                              at_t = a3p.tile([128, N], bf16, tag="at_t")
                              nc.vector.tensor_scalar(
                                  at_t, e, r_c, None, op0=AOP.mult,
                              )
                              # u8 = attn * attn3f  [Pool TT, fp8 out]
                              nc.gpsimd.tensor_tensor(
                                  u8[:, at, :], at_t, a3, op=AOP.mult,
                              )
                          else:
                              # u8 = (e*r)*attn3f  [DVE fused stt, fp8 out]
                              nc.vector.scalar_tensor_tensor(
                                  u8[:, at, :], e, r_c, a3,
                                  op0=AOP.mult, op1=AOP.mult,
                              )
                      else:
                          # tile 6: rows 0..31 are feature rows 768..799;
                          # partition 32 is the preset proj-bias row.
                          at_t = a3p.tile([128, N], bf16, tag="at_t")
                          a3 = a3p.tile([128, N], bf16, tag="a3")
                          nc.vector.tensor_scalar(
                              at_t[0:32, :], e[0:32, :], r_c[0:32, :], None,
                              op0=AOP.mult,
                          )
                          nc.vector.tensor_scalar(
                              a3[0:32, :], e[0:32, :], r_c[0:32, :], 3.0,
                              op0=AOP.mult, op1=AOP.add,
                          )
                          nc.vector.tensor_tensor(
                              ua6[0:32, :], at_t[0:32, :], a3[0:32, :],
                              op=AOP.mult,
                          )

                  prev = (b, u_v, u8)
                  cur_s1 = nxt_s1

              out_nat = outp.tile([128, 8, DIM], f32, tag="out_nat")
              pstate = (*prev, out_nat)
              for pair in range(4):
                  emit_proj_pair(pstate, pair)

    nc.compile()
    return nc


_PROGRAM_CACHE = {}


def _get_program():
    if "nc" not in _PROGRAM_CACHE:
        _PROGRAM_CACHE["nc"] = _build_program()
    return _PROGRAM_CACHE["nc"]


def _f8(x):
    return np.asarray(x, dtype=ml_dtypes.float8_e4m3)


def _prepare_host_inputs(x, text, kv_w, kv_g, kv_b, kv_m, kv_v,
                         q_w, q_g, q_b, q_m, q_v,
                         proj_w, proj_g, proj_b, proj_m, proj_v,
                         biases, H, W):
    H, W = int(H), int(W)
    scale = KD ** -0.5

    kv_we, kv_be = _fold_bn(np.asarray(kv_w), np.asarray(kv_g), np.asarray(kv_b),
                            np.asarray(kv_m), np.asarray(kv_v))
    q_we, q_be = _fold_bn(np.asarray(q_w), np.asarray(q_g), np.asarray(q_b),
                          np.asarray(q_m), np.asarray(q_v))
    p_we, p_be = _fold_bn(np.asarray(proj_w), np.asarray(proj_g), np.asarray(proj_b),
                          np.asarray(proj_m), np.asarray(proj_v))

    # kv feature permutation: k rows first (h-major kd), then v rows (h-major d)
    k_src = np.array([h * (KD + D) + j for h in range(NH) for j in range(KD)])
    v_src = np.array([h * (KD + D) + KD + d for h in range(NH) for d in range(D)])
    perm = np.concatenate([k_src, v_src])
    w1 = kv_we[perm] * W1S                # [640, 256], pre-scaled
    b1 = kv_be[perm]                      # [640]
    w1t = np.ascontiguousarray(
        w1.T.reshape(2, 128, H_KV).transpose(1, 0, 2)
    ).astype(np.float32)                  # [128, 2, 640]
    w8h_host = _f8(w1t)
    w8l_host = _f8(w1t - w8h_host.astype(np.float32))
    b1v_host = np.ascontiguousarray(
        b1[NH_KD:].reshape(N_VTILES, 128).T
    ).astype(np.float32)                  # [128, 4]

    # q on host (tiny), scaled
    q = (np.asarray(text, np.float32) @ q_we.T + q_be).reshape(NT, NH, KD)
    q = (q * scale).astype(np.float32)

    # block-diagonal lhsT for the attn matmul: [128 (h,kd), 896 (h,t)]
    qlhs_host = np.zeros((128, NT_PAD), np.float32)
    rows = np.arange(NH * NT)
    hh, tt = rows // NT, rows % NT
    for kd in range(KD):
        qlhs_host[hh * KD + kd, rows] = q[tt, hh, kd]
    qlhs_host = qlhs_host.astype(ml_dtypes.bfloat16)

    # bias table[(h,t), n] = biases[h, idx[t, n]] + q~[t,h] . b1_k[h]
    t_i = np.arange(NT)
    n_i = np.arange(N)
    p1x, p1y = t_i // 100, t_i % 100
    p2x, p2y = n_i // W, n_i % W
    idx = (np.abs(p1x[:, None] - p2x[None, :]) * 100
           + np.abs(p1y[:, None] - p2y[None, :]))        # [100, N]
    bias_g = np.asarray(biases, np.float32)[:, idx]       # [NH, 100, N]
    b1k = b1[:NH_KD].reshape(NH, KD)                      # [8, 16]
    cq = np.einsum("thk,hk->ht", q, b1k)                  # [8, 100]
    bg2_full = np.zeros((NT_PAD, N), np.float32)
    bg2_full[: NH * NT] = (bias_g + cq[:, :, None]).reshape(NH * NT, N)
    bg2_t = np.ascontiguousarray(
        bg2_full.reshape(N_ATILES, 128, N).transpose(1, 0, 2)
    )                                                     # [128, 7, N]
    # hi/lo fp8 split for the DoubleRow inject; exp() table for m-route tiles
    bgh = _f8(bg2_t)
    bgl = _f8(bg2_t - bgh.astype(np.float32))
    bgt8_host = np.ascontiguousarray(
        np.stack([bgh, bgl], axis=2)
    )                                                     # [128, 7, 2, N] fp8
    if M_ROUTE:
        ebt_host = np.ascontiguousarray(
            np.exp(bg2_t[:, list(M_ROUTE), :])
        ).astype(ml_dtypes.bfloat16)
    else:
        ebt_host = np.zeros((128, 1, N), ml_dtypes.bfloat16)

    # proj weights.
    # bf16 group (x OSC): v rows (u_v = 6*hswish(v), /6) then attn tile 6
    # (feature rows 768..799 at partitions 0..31, proj bias at partition 32).
    wpv_host = np.zeros((128, N_VTILES + 1, DIM), np.float32)
    wpv_host[:, :N_VTILES, :] = (
        p_we[:, :DH].T.reshape(N_VTILES, 128, DIM).transpose(1, 0, 2)
        * (OSC / 6.0)
    )
    wpv_host[0:32, N_VTILES, :] = p_we[:, DH + 768 : DH + 800].T * (OSC / 6.0)
    wpv_host[32, N_VTILES, :] = p_be * OSC
    wpv_host = wpv_host.astype(ml_dtypes.bfloat16)

    # fp8 group: attn feature rows 0..767; u8 = U8S*u_a, wpa8 = W8S*w/6
    wpa8_host = _f8(
        np.ascontiguousarray(
            (p_we[:, DH : DH + 768].T * (W8S / 6.0))
            .reshape(N_FP8, 128, DIM).transpose(1, 0, 2)
        )
    )                                                     # [128, 6, 256]

    return {
        "w8h": w8h_host,
        "w8l": w8l_host,
        "qlhs": qlhs_host,
        "bgt8": bgt8_host,
        "ebt": ebt_host,
        "wpv": wpv_host,
        "wpa8": wpa8_host,
        "b1v": b1v_host,
    }


def kernel(**inputs):
    x = np.asarray(inputs["x"], np.float32)
    consts = _prepare_host_inputs(**inputs)
    # x.T per batch, f-major tiles, fp8 hi/lo: [B, 2, 128, N]
    xt_all = np.ascontiguousarray(x.transpose(0, 2, 1).reshape(B, 2, 128, N))
    x8h_all = _f8(xt_all)
    x8l_all = _f8(xt_all - x8h_all.astype(np.float32))

    nc = _get_program()
    in_maps = []
    for c in range(NCORES):
        m = dict(consts)
        m["x8h"] = np.ascontiguousarray(x8h_all[c * BLOC : (c + 1) * BLOC])
        m["x8l"] = np.ascontiguousarray(x8l_all[c * BLOC : (c + 1) * BLOC])
        in_maps.append(m)

    trace = bool(int(os.environ.get("KERNEL_TRACE", "0")))
    res = None
    last_err = None
    for _attempt in range(3):
        try:
            res = run_bass_kernel_spmd(
                nc, in_maps, core_ids=list(range(NCORES)), trace=trace
            )
            break
        except Exception as e:  # transient NRT device wedge: retry
            last_err = e
            trace = False  # trace path may be unavailable (no ntff hook)
    if res is None:
        raise last_err
    if trace and res.exec_time_ns is not None:
        print(f"HW exec time: {res.exec_time_ns} ns")
        if res.instructions_and_trace is not None:
            print(f"trace: {res.instructions_and_trace[1]}")
    out = np.concatenate([r["out"] for r in res.results], axis=0)
    return out
